# revision 27
# baseline (speedup 1.0000x reference)
"""Causal cross-attention kernel for 8 TRN2 NeuronCores.

Sharding: data-parallel over batch (B=2) x tensor-parallel over head
groups (16 heads -> 4 groups of 4). Core c handles batch c//4, heads
[4*(c%4), 4*(c%4)+4). Each core computes its partial output projection
(w_out rows for its heads); the host sums the 4 partials per batch
(the "all-reduce"), adds b_out, and fixes the fully-masked row 0.

Device dataflow per core:
  qT[f',s] = sum_f wq[f,f'] xT_from[f,s]     (lhsT=wq tile, rhs=xT tile)
  kT[f',s] = sum_f wk[f,f'] xT_to[f,s]       (stored per-head, padded to
                                              K=128 with zero partitions)
  v[z,f']  = sum_f xT_to[f,z] wv[f,f']       (stored padded to 128 cols:
                                              col 64 = ones -> denominator,
                                              cols 65..127 zero)
  scoresT[z,s] = kT_pad.T qT  per head       (K=128 full PE array)
  P = exp(scoresT + tri_mask)                (mask added in-place in PSUM
                                              on the 128-col diagonal band)
  out'T[*,s] = v'.T P                        (row 64 = softmax denominator)
  attn_outT = out'T[0:64] * (1/out'T[64])    (PE-broadcast of recip row)
  out[s,fo] = sum_hd attn_outT[hd,s] wo[hd,fo]

Scheduling: a single fused emission stream. Scores run LEAD blocks ahead
of their AV matmuls so the exp (Act engine) latency is hidden; projection
and output-projection matmuls are queued as "filler" chiplets and popped
into the PE stream during exp-paced stretches, keeping the PE array busy
(HAM un-throttled at 2.4GHz) and all engines overlapped.
"""

import numpy as np
import concourse.bass as bass
import concourse.mybir as mybir
import concourse.tile as tile
from concourse.bass_utils import run_bass_kernel_spmd

B, S, F, H = 2, 2048, 1024, 16
NCORES = 8
HG = 4          # head groups (tensor-parallel degree per batch)
HPC = H // HG   # heads per core = 4
D = F // H      # head dim = 64
CW = HPC * D    # per-core projection width = 256
MASK_VAL = 1.0e12
SC = 512        # s-chunk for projections / scores
NZC = S // 128  # 16 z-chunks
LEAD = 3        # score blocks emitted ahead of their AV matmul

f32 = mybir.dt.float32
f32r = mybir.dt.float32r
bf16 = mybir.dt.bfloat16

# Walrus encodes at most 1 sync wait on most TRN2 instructions; Tile can
# attach several. Redistribute excess waits onto preceding same-engine NOPs.


def _split_excess_waits(nc):
    for fn in nc.m.functions:
        for bb in fn.blocks:
            insts = list(bb.instructions)
            out = []
            changed = False
            for inst in insts:
                si = inst.sync_info
                waits = list(si.on_wait) if si is not None else []
                if len(waits) > 1:
                    changed = True
                    inst.sync_info = mybir.SyncInfo(
                        on_update=list(si.on_update), on_wait=waits[-1:]
                    )
                    for idx, w in enumerate(waits[:-1]):
                        nop = mybir.InstNoOp(name=f"{inst.name}-wsplit{idx}")
                        nop.engine = inst.engine
                        nop.sync_info = mybir.SyncInfo(on_update=[], on_wait=[w])
                        out.append(nop)
                out.append(inst)
            if changed:
                bb.instructions = out


def _round_f32r(x):
    u = np.ascontiguousarray(x, dtype=np.float32).view(np.uint32)
    u = ((u.astype(np.uint64) + 0x1000) & 0xFFFFE000).astype(np.uint32)
    return u.view(np.float32)


def _build():
    nc = bass.Bass()
    xf_d = nc.declare_dram_parameter("xf", [F, S], f32r, isOutput=False)
    xt_d = nc.declare_dram_parameter("xt", [F, S], f32r, isOutput=False)
    wq_d = nc.declare_dram_parameter("wq", [F, CW], f32r, isOutput=False)
    wk_d = nc.declare_dram_parameter("wk", [F, CW], f32r, isOutput=False)
    wv_d = nc.declare_dram_parameter("wv", [F, CW], f32r, isOutput=False)
    wo_d = nc.declare_dram_parameter("wo", [CW, F], bf16, isOutput=False)
    bq_d = nc.declare_dram_parameter("bq", [CW, 1], f32, isOutput=False)
    bk_d = nc.declare_dram_parameter("bk", [CW, 1], f32, isOutput=False)
    bv_d = nc.declare_dram_parameter("bv", [1, CW], f32r, isOutput=False)
    tri_d = nc.declare_dram_parameter("tri", [128, 128], bf16, isOutput=False)
    out_d = nc.declare_dram_parameter("out", [S, F], f32, isOutput=True)

    nsc = S // SC  # 4

    with tile.TileContext(nc) as tc:
        with (
            tc.tile_pool(name="const", bufs=1) as cpool,
            tc.tile_pool(name="xf", bufs=2) as xfpool,
            tc.tile_pool(name="xt", bufs=2) as xtpool,
            tc.tile_pool(name="work", bufs=2) as wpool,
            tc.tile_pool(name="pbuf", bufs=6) as ppool,
            tc.tile_pool(name="outst", bufs=2) as opool,
            tc.tile_pool(name="aop", bufs=8) as apool,
            tc.tile_pool(name="ps_sc", bufs=3, space="PSUM") as ps_sc,
            tc.tile_pool(name="ps_av", bufs=3, space="PSUM") as ps_av,
            tc.tile_pool(name="ps_pr", bufs=2, space="PSUM") as ps_pr,
        ):
            # ---- persistent tiles ----
            wq = cpool.tile([128, 8, CW], f32r)
            wk = cpool.tile([128, 8, CW], f32r)
            wv = cpool.tile([128, 8, CW], f32r)
            wo = cpool.tile([128, 2, F], bf16)
            bq = cpool.tile([128, 2, 1], f32)
            bk = cpool.tile([128, 2, 1], f32)
            bv = cpool.tile([1, CW], f32r)
            tri = cpool.tile([128, 128], bf16)
            ones1 = cpool.tile([1, 128], f32r)
            qT = cpool.tile([128, 2, S], bf16)
            kT = cpool.tile([128, HPC, S], bf16)
            v1 = cpool.tile([128, NZC, HPC, D + 1], bf16)
            aoT = cpool.tile([128, 2, S], bf16)

            ones_f = cpool.tile([1, 128], f32)
            nc.vector.memset(ones_f[:], 1.0)
            nc.vector.tensor_copy(ones1[:], ones_f[:])  # f32r cast
            epsv = cpool.tile([1, D + 1], bf16)
            nc.vector.memset(epsv[:], 0.0)
            nc.vector.memset(epsv[:, D : D + 1], 1.0e-30)
            ones_b = cpool.tile([1, 128], bf16)
            nc.vector.memset(ones_b[:], 1.0)
            nc.gpsimd.memset(v1[:, :, :, D : D + 1], 1.0)
            for h in range(HPC):
                zpo = 64 - 64 * (h % 2)
                nc.gpsimd.memset(kT[zpo : zpo + 64, h, :], 0.0)

            # ---- startup DMAs (interleaved so proj(0) starts ASAP) ----
            xtiles = [None] * nsc

            def load_x(sc):
                xf = xfpool.tile([128, 8, SC], f32r, tag="xf")
                xt = xtpool.tile([128, 8, SC], f32r, tag="xt")
                s0 = sc * SC
                for k in range(8):
                    nc.sync.dma_start(
                        out=xf[:, k, :], in_=xf_d[128 * k : 128 * (k + 1), s0 : s0 + SC]
                    )
                    nc.sync.dma_start(
                        out=xt[:, k, :], in_=xt_d[128 * k : 128 * (k + 1), s0 : s0 + SC]
                    )
                xtiles[sc] = (xf, xt)

            for k in range(8):
                nc.sync.dma_start(
                    out=wq[:, k, :], in_=wq_d[128 * k : 128 * (k + 1), :]
                )
            nc.sync.dma_start(
                out=bq[:], in_=bq_d[:].rearrange("(m p) c -> p m c", p=128)
            )
            load_x(0)
            for k in range(8):
                nc.sync.dma_start(
                    out=wk[:, k, :], in_=wk_d[128 * k : 128 * (k + 1), :]
                )
            nc.sync.dma_start(
                out=bk[:], in_=bk_d[:].rearrange("(m p) c -> p m c", p=128)
            )
            for k in range(8):
                nc.sync.dma_start(
                    out=wv[:, k, :], in_=wv_d[128 * k : 128 * (k + 1), :]
                )
            nc.sync.dma_start(out=bv[:], in_=bv_d[:])
            nc.sync.dma_start(out=tri[:], in_=tri_d[:])
            nc.sync.dma_start(
                out=wo[:], in_=wo_d[:].rearrange("(m p) c -> p m c", p=128)
            )
            load_x(1)

            # ---- filler machinery ----
            # queue of (tag, closure): tag = s-chunk index that REQUIRES this
            # work to be complete before its attention groups start (99 =
            # output projection, only data-dependent, never force-required).
            filler_q = []

            def pop_filler(n=1):
                for _ in range(n):
                    if not filler_q:
                        return
                    _, fn = filler_q.pop(0)
                    fn()

            def force_drain(tag):
                while filler_q and filler_q[0][0] <= tag:
                    _, fn = filler_q.pop(0)
                    fn()

            # ---- projection chiplets for one s-chunk ----
            def enqueue_proj(sc):
                s0 = sc * SC

                def qk_chain(which, m):
                    st = {}

                    def step(k0, st=st, which=which, m=m):
                        xf, xt = xtiles[sc]
                        if "t" not in st:
                            st["t"] = ps_pr.tile([128, SC], f32, tag="pr", name="prt")
                        w_, x_ = (wq, xf) if which == "q" else (wk, xt)
                        for k in (k0, k0 + 1):
                            nc.tensor.matmul(
                                st["t"],
                                w_[:, k, m * 128 : (m + 1) * 128],
                                x_[:, k, :],
                                start=(k == 0),
                                stop=(k == 7),
                            )
                        if k0 == 6:
                            if which == "q":
                                nc.vector.tensor_scalar_add(
                                    qT[:, m, s0 : s0 + SC],
                                    st["t"][:],
                                    bq[:, m, :],
                                )
                            else:
                                for j in range(2):
                                    po = 64 * j
                                    nc.vector.tensor_scalar_add(
                                        kT[po : po + 64, 2 * m + j, s0 : s0 + SC],
                                        st["t"][po : po + 64, :],
                                        bk[po : po + 64, m, :],
                                    )

                    for k0 in range(0, 8, 2):
                        filler_q.append((sc, lambda k0=k0: step(k0)))

                def v_chain(zz):
                    st = {}
                    zc = sc * (SC // 128) + zz

                    def step(k0, st=st, zz=zz, zc=zc):
                        xf, xt = xtiles[sc]
                        if "t" not in st:
                            st["t"] = ps_pr.tile([128, SC], f32, tag="pr", name="prt")
                        pv = st["t"]
                        for k in (k0, k0 + 1):
                            nc.tensor.matmul(
                                pv[:, :CW],
                                xt[:, k, zz * 128 : (zz + 1) * 128],
                                wv[:, k, :],
                                start=(k == 0),
                                stop=False,
                            )
                        if k0 == 6:
                            nc.tensor.matmul(
                                pv[:, :CW], ones1[:, :], bv[:], start=False, stop=True
                            )
                            nc.scalar.copy(
                                v1[:, zc, :, 0:D],
                                pv[:, :CW].rearrange("p (h d) -> p h d", h=HPC),
                            )

                    for k0 in range(0, 8, 2):
                        filler_q.append((sc, lambda k0=k0: step(k0)))

                for m in range(2):
                    qk_chain("q", m)
                    qk_chain("k", m)
                for zz in range(SC // 128):
                    v_chain(zz)

            # ---- output projection chiplets for one s-chunk ----
            def enqueue_outproj(sc):
                for so in range(sc * 4, sc * 4 + 4):
                    s0 = so * 128
                    st = {}

                    def step(fo, st=st, s0=s0):
                        if "ost" not in st:
                            st["ost"] = opool.tile([128, F], f32, tag="ost", name="ost")
                        po_ = ps_pr.tile([128, SC], f32, tag="pr")
                        for m in range(2):
                            nc.tensor.matmul(
                                po_[:],
                                aoT[:, m, s0 : s0 + 128],
                                wo[:, m, fo * SC : (fo + 1) * SC],
                                start=(m == 0),
                                stop=(m == 1),
                            )
                        if fo == 0:
                            nc.vector.tensor_copy(
                                st["ost"][:, fo * SC : (fo + 1) * SC], po_[:]
                            )
                        else:
                            nc.scalar.copy(
                                st["ost"][:, fo * SC : (fo + 1) * SC], po_[:]
                            )
                        if fo == 1:
                            nc.sync.dma_start(
                                out=out_d[s0 : s0 + 128, :], in_=st["ost"][:]
                            )

                    for fo in range(2):
                        filler_q.append((99, lambda fo=fo, step=step: step(fo)))

            # ---- fused emission ----
            enqueue_proj(0)
            enqueue_proj(1)
            force_drain(0)
            enqueue_proj(2)

            for sc in range(nsc):
                s0 = sc * SC
                force_drain(sc)
                if sc == 1:
                    enqueue_proj(3)
                ao_pairs = {}
                for h in range(HPC):
                    m, po = divmod(h, 2)
                    po *= 64
                    # AV in [s, d] orientation: per s-block (sb), accumulate
                    # over z-blocks with P as the stationary operand. Column
                    # 64 (ones in v1) accumulates the softmax denominator,
                    # one per PARTITION, so the reciprocal is per-partition
                    # and cheap on DVE.
                    pav = ps_av.tile([128, 4, D + 1], f32, tag="pav")
                    nz = (sc + 1) * (SC // 128)
                    ptiles = [None] * nz

                    def do_av(zc, pav=pav, h=h, sc=sc, ptiles=ptiles):
                        for sb in range(max(0, zc - 4 * sc), 4):
                            nc.tensor.matmul(
                                pav[:, sb, :],
                                ptiles[zc][:, sb * 128 : (sb + 1) * 128],
                                v1[:, zc, h, 0 : D + 1],
                                start=(zc == 0),
                                stop=(zc == 4 * sc + sb)
                                and not (sc == 0 and sb == 0),
                                skip_group_check=True,
                            )
                            if sc == 0 and sb == 0 and zc == 0:
                                # epsilon so the all-masked column s=0 gets a
                                # nonzero denominator (no inf/NaN downstream)
                                nc.tensor.matmul(
                                    pav[:, 0, :], ones_b[:, :], epsv[:],
                                    start=False, stop=True,
                                    skip_group_check=True,
                                )

                    for zc in range(nz):
                        z0 = zc * 128
                        off = max(0, z0 - s0)
                        ps = ps_sc.tile([128, SC], f32, tag="ps")
                        nc.tensor.matmul(
                            ps[:, off:SC],
                            kT[:, h, z0 : z0 + 128],
                            qT[:, m, s0 + off : s0 + SC],
                            start=True,
                            stop=True,
                        )
                        p = ppool.tile([128, SC], bf16, tag="p")
                        ptiles[zc] = p
                        nc.scalar.activation(
                            p[:, off:SC], ps[:, off:SC],
                            mybir.ActivationFunctionType.Exp,
                        )
                        if off or zc * 128 == s0:
                            # diagonal band: zero the masked (z >= s) region
                            # post-exp with a 0/1 triangle (bf16, SBUF-only,
                            # runs on the otherwise-idle gpsimd engine)
                            nc.gpsimd.tensor_tensor(
                                out=p[:, off : off + 128],
                                in0=p[:, off : off + 128],
                                in1=tri[:],
                                op=mybir.AluOpType.mult,
                            )
                        if zc >= LEAD:
                            do_av(zc - LEAD)
                            pop_filler()
                    for zc in range(max(0, nz - LEAD), nz):
                        do_av(zc)
                        pop_filler()
                    # normalize: per-partition reciprocal of the denominator
                    # column, multiply into the head's half of the ao pair
                    recip4 = wpool.tile([128, 4], f32, tag="recip4")
                    for sb in range(4):
                        nc.vector.reciprocal(
                            recip4[:, sb : sb + 1], pav[:, sb, D : D + 1]
                        )
                    for sb in range(4):
                        if h % 2 == 0:
                            ao_pairs[sb] = apool.tile(
                                [128, 128], bf16, tag="aop", name="aop"
                            )
                        nc.vector.tensor_scalar_mul(
                            ao_pairs[sb][:, po : po + D],
                            pav[:, sb, 0:D],
                            recip4[:, sb : sb + 1],
                        )
                        if h % 2 == 1:
                            # both heads of m-block done: transpose [s, hd]
                            # -> aoT [hd, s] via the DMA transpose XBAR
                            nc.sync.dma_start(
                                out=aoT[:, m, s0 + sb * 128 : s0 + (sb + 1) * 128],
                                in_=ao_pairs[sb][:],
                                transpose=True,
                            )
                enqueue_outproj(sc)
                if sc < 2:
                    load_x(sc + 2)

            while filler_q:
                pop_filler()

    _split_excess_waits(nc)
    return nc


_CACHE = {}


def _get_nc():
    if "nc" not in _CACHE:
        _CACHE["nc"] = _build()
    return _CACHE["nc"]


def _ensure_ntff_hook():
    """The agent image's antenv lacks axon_hooks, so run_bass_kernel_spmd's
    trace path can't import it. Synthesize the module and install the
    ctypes NTFF hook from trn_agent_boot (same thing boot() would do)."""
    import sys
    import types

    if "antenv.axon_hooks" not in sys.modules:
        mod = types.ModuleType("antenv.axon_hooks")
        holder = [None]
        mod.set_axon_ntff_profile_hook = lambda h: holder.__setitem__(0, h)
        mod.get_axon_ntff_profile_hook = lambda: holder[0]
        sys.modules["antenv.axon_hooks"] = mod
        import antenv

        antenv.axon_hooks = mod
    import antenv.axon_hooks as ah

    if ah.get_axon_ntff_profile_hook() is None:
        try:
            from trn_agent_boot.trn_boot import _ntff_profile_via_ctypes

            ah.set_axon_ntff_profile_hook(
                _ntff_profile_via_ctypes("/opt/axon/libaxon_pjrt.so")
            )
        except Exception:
            pass


import ml_dtypes

_bf16np = ml_dtypes.bfloat16


def _host_tri():
    i = np.arange(128)[:, None]
    c = np.arange(128)[None, :]
    return (i < c).astype(_bf16np)


def kernel(attend_from, attend_to, w_q, b_q, w_kv, b_kv, w_out, b_out, _trace=False):
    attend_from = np.asarray(attend_from, dtype=np.float32)
    attend_to = np.asarray(attend_to, dtype=np.float32)
    w_q = np.asarray(w_q, dtype=np.float32)
    b_q = np.asarray(b_q, dtype=np.float32)
    w_kv = np.asarray(w_kv, dtype=np.float32)
    b_kv = np.asarray(b_kv, dtype=np.float32)
    w_out = np.asarray(w_out, dtype=np.float32)
    b_out = np.asarray(b_out, dtype=np.float32)

    tri = _host_tri()
    xT = [_round_f32r(attend_from[b].T) for b in range(B)]
    xTt = [_round_f32r(attend_to[b].T) for b in range(B)]

    in_maps = []
    for c in range(NCORES):
        b, hg = divmod(c, HG)
        cols = slice(hg * CW, (hg + 1) * CW)
        in_maps.append(
            {
                "xf": xT[b],
                "xt": xTt[b],
                "wq": _round_f32r(w_q[:, cols]),
                "wk": _round_f32r(w_kv[:, cols]),
                "wv": _round_f32r(w_kv[:, F:][:, cols]),
                "wo": w_out[cols, :].astype(_bf16np),
                "bq": np.ascontiguousarray(b_q[cols].reshape(CW, 1)),
                "bk": np.ascontiguousarray(b_kv[cols].reshape(CW, 1)),
                "bv": _round_f32r(b_kv[F:][cols].reshape(1, CW)),
                "tri": tri,
                "out": np.zeros((S, F), np.float32),
            }
        )

    nc = _get_nc()
    if _trace:
        _ensure_ntff_hook()
    res = run_bass_kernel_spmd(nc, in_maps, list(range(NCORES)), trace=_trace)

    out = np.zeros((B, S, F), np.float64)
    for c in range(NCORES):
        b = c // HG
        out[b] += res.results[c]["out"].astype(np.float64)
    out += b_out.astype(np.float64)[None, None, :]

    # Row 0 of the reference is fully masked -> softmax is exactly uniform
    # over all Z positions; compute it directly on the host.
    w_v = w_kv[:, F:].astype(np.float64)
    for b in range(B):
        val_mean = attend_to[b].astype(np.float64).mean(axis=0) @ w_v + b_kv[
            F:
        ].astype(np.float64)
        out[b, 0, :] = val_mean @ w_out.astype(np.float64) + b_out.astype(np.float64)

    if _trace:
        kernel._last_result = res
    return out.astype(np.float32)
